# revision 3
# baseline (speedup 1.0000x reference)
"""DBM mean-field Gibbs sampler on 8 trn2 NeuronCores.

Strategy (model-parallel over the hidden dim, NOT the batch):
  - Each core c owns a 512-row slice H_c of both hidden layers.
  - Weight slices W2[:, H_c], W2.T[:, H_c] live in SBUF in fp16 for the
    whole kernel (data-parallel would stream 128 MB of replicated weights
    per core per iteration -> hopelessly HBM-bound at ~3.6 ms).
  - States are kept transposed (s.T, shape [4096, 512]) so every matmul is
    lhsT[k-tile of W slice] @ rhs[k-tile of state, full batch]: full
    128x128 PE array, N=512/256 moving columns.
  - data @ W1 + b1 is loop-invariant: precomputed once into C1 (fp32).
  - After each half-step every core AllGathers its new state slice
    (fp16, batch-chunked so the collective overlaps the other chunk's
    matmuls on the PE).
  - Accumulation in fp32 PSUM, sigmoid in fp32 on the scalar engine;
    only matmul operands are fp16 (max rel err vs fp32 ref ~6e-4).

Outputs per core: its own fp32 slice of s1.T / s2.T; the host
concatenates and transposes back.
"""

import numpy as np

B = 512
D = 4096
NCORES = 8
SLICE = D // NCORES  # 512
P = 128
KT = D // P          # 32 k-tiles over the contraction dim
MT = SLICE // P      # 4 m-tiles of the owned slice
CHUNKS = 2           # batch chunks per half-step (AG/computation overlap)

_CACHE = {}


def _build(iters: int, local_sim: bool = False):
    import concourse.bacc as bacc
    import concourse.mybir as mybir
    import concourse.tile as tile

    f16 = mybir.dt.float16
    f32 = mybir.dt.float32
    SIG = mybir.ActivationFunctionType.Sigmoid
    IDENT = mybir.ActivationFunctionType.Identity
    BYPASS = mybir.AluOpType.bypass

    CB = B // CHUNKS
    RG = [list(range(NCORES))]

    nc = bacc.Bacc(
        "TRN2",
        target_bir_lowering=False,
        debug=False,
        enable_asserts=False,
        num_devices=NCORES,
    )

    w2t_d = nc.dram_tensor("w2t", [D, SLICE], f16, kind="ExternalInput")
    w2_d = nc.dram_tensor("w2", [D, SLICE], f16, kind="ExternalInput")
    w1_d = nc.dram_tensor("w1", [D, SLICE], f16, kind="ExternalInput")
    dataT_d = nc.dram_tensor("dataT", [D, B], f16, kind="ExternalInput")
    b1s_d = nc.dram_tensor("b1s", [SLICE], f32, kind="ExternalInput")
    b2s_d = nc.dram_tensor("b2s", [SLICE], f32, kind="ExternalInput")
    b2f_d = nc.dram_tensor("b2f", [D], f32, kind="ExternalInput")
    s1_out_d = nc.dram_tensor("s1_out", [SLICE, B], f32, kind="ExternalOutput")
    s2_out_d = nc.dram_tensor("s2_out", [SLICE, B], f32, kind="ExternalOutput")

    with tile.TileContext(nc) as tc:
        with (
            tc.tile_pool(name="persist", bufs=1) as persist,
            tc.tile_pool(name="psum", bufs=8, space="PSUM") as psum_pool,
            tc.tile_pool(name="agin_pool", bufs=2, space="DRAM") as agin_pool,
            tc.tile_pool(name="agout_pool", bufs=2, space="DRAM") as agout_pool,
        ):
            w2t_sb = persist.tile([P, KT, SLICE], f16, name="w2t_sb")
            w2_sb = persist.tile([P, KT, SLICE], f16, name="w2_sb")
            s1_sb = persist.tile([P, KT, B], f16, name="s1_sb")
            s2_sb = persist.tile([P, KT, B], f16, name="s2_sb")
            c1_sb = persist.tile([P, MT, B], f32, name="c1_sb")
            b1s_sb = persist.tile([P, MT], f32, name="b1s_sb")
            b2s_sb = persist.tile([P, MT], f32, name="b2s_sb")
            b2f_sb = persist.tile([P, KT], f32, name="b2f_sb")
            zeros_sb = persist.tile([P, B], f16, name="zeros_sb")

            nc.sync.dma_start(
                w2t_sb[:], w2t_d.ap().rearrange("(k p) m -> p k m", p=P)
            )
            nc.sync.dma_start(
                w2_sb[:], w2_d.ap().rearrange("(k p) m -> p k m", p=P)
            )
            nc.sync.dma_start(b1s_sb[:], b1s_d.ap().rearrange("(m p) -> p m", p=P))
            nc.sync.dma_start(b2s_sb[:], b2s_d.ap().rearrange("(m p) -> p m", p=P))
            nc.sync.dma_start(b2f_sb[:], b2f_d.ap().rearrange("(k p) -> p k", p=P))
            nc.vector.memset(zeros_sb[:], 0.0)

            # s2^0 = sigmoid(b2) broadcast over the batch (s1^0 is never read).
            for k in range(KT):
                nc.scalar.activation(
                    s2_sb[:, k, :], zeros_sb[:], SIG, bias=b2f_sb[:, k : k + 1]
                )

            # C1 = b1[H_c] + (W1.T @ data.T)[H_c]  (loop-invariant, fp32)
            with (
                tc.tile_pool(name="dataT_pool", bufs=1) as dataT_pool,
                tc.tile_pool(name="w1_pool", bufs=2) as w1_pool,
            ):
                dataT_sb = dataT_pool.tile([P, KT, B], f16, name="dataT_sb")
                nc.sync.dma_start(
                    dataT_sb[:], dataT_d.ap().rearrange("(k p) b -> p k b", p=P)
                )
                for m in range(MT):
                    w1_blk = w1_pool.tile([P, KT, P], f16, name="w1_blk", tag="w1b")
                    nc.sync.dma_start(
                        w1_blk[:],
                        w1_d.ap()[:, m * P : (m + 1) * P].rearrange(
                            "(k p) m -> p k m", p=P
                        ),
                    )
                    ps = psum_pool.tile([P, B], f32, name="ps_pre", tag="ps")
                    for k in range(KT):
                        nc.tensor.matmul(
                            ps[:],
                            w1_blk[:, k, :],
                            dataT_sb[:, k, :],
                            start=(k == 0),
                            stop=(k == KT - 1),
                        )
                    nc.scalar.activation(
                        c1_sb[:, m, :], ps[:], IDENT, bias=b1s_sb[:, m : m + 1]
                    )

            with tc.tile_pool(name="stage", bufs=3) as stage:
                for it in range(iters):
                    last = it == iters - 1
                    # ---- half A: s1[H_c] = sigmoid(C1 + (W2 @ s2T)[H_c]) ----
                    for ch in range(CHUNKS):
                        cs = slice(ch * CB, (ch + 1) * CB)
                        own = stage.tile([P, MT, CB], f16, name="own", tag="own")
                        for m in range(MT):
                            ps = psum_pool.tile([P, CB], f32, name="ps", tag="ps")
                            for k in range(KT):
                                nc.tensor.matmul(
                                    ps[:],
                                    w2t_sb[:, k, m * P : (m + 1) * P],
                                    s2_sb[:, k, cs],
                                    start=(k == 0),
                                    stop=(k == KT - 1),
                                )
                            nc.vector.tensor_add(ps[:], ps[:], c1_sb[:, m, cs])
                            nc.scalar.activation(own[:, m, :], ps[:], SIG)
                            if last:
                                outst = stage.tile(
                                    [P, CB], f32, name="outst", tag="outst"
                                )
                                nc.scalar.activation(outst[:], ps[:], SIG)
                                nc.sync.dma_start(
                                    s1_out_d.ap()[m * P : (m + 1) * P, cs], outst[:]
                                )
                        agin = agin_pool.tile([SLICE, CB], f16, name="agin", tag="agin")
                        agout = agout_pool.tile(
                            [D, CB], f16, name="agout", tag="agout", addr_space="Shared"
                        )
                        nc.sync.dma_start(
                            agin[:].rearrange("(m p) b -> p m b", p=P), own[:]
                        )
                        if not local_sim:
                            nc.gpsimd.collective_compute(
                                "AllGather",
                                BYPASS,
                                replica_groups=RG,
                                ins=[agin[:]],
                                outs=[agout[:]],
                            )
                        nc.sync.dma_start(
                            s1_sb[:, :, cs],
                            agout[:].rearrange("(k p) b -> p k b", p=P),
                        )

                    # ---- half B: s2[H_c] = sigmoid(b2[H_c] + (W2.T @ s1T)[H_c]) ----
                    for ch in range(CHUNKS):
                        cs = slice(ch * CB, (ch + 1) * CB)
                        own = (
                            None
                            if last
                            else stage.tile([P, MT, CB], f16, name="own", tag="own")
                        )
                        for m in range(MT):
                            ps = psum_pool.tile([P, CB], f32, name="ps", tag="ps")
                            for k in range(KT):
                                nc.tensor.matmul(
                                    ps[:],
                                    w2_sb[:, k, m * P : (m + 1) * P],
                                    s1_sb[:, k, cs],
                                    start=(k == 0),
                                    stop=(k == KT - 1),
                                )
                            if last:
                                outst = stage.tile(
                                    [P, CB], f32, name="outst", tag="outst"
                                )
                                nc.scalar.activation(
                                    outst[:], ps[:], SIG, bias=b2s_sb[:, m : m + 1]
                                )
                                nc.sync.dma_start(
                                    s2_out_d.ap()[m * P : (m + 1) * P, cs], outst[:]
                                )
                            else:
                                nc.scalar.activation(
                                    own[:, m, :],
                                    ps[:],
                                    SIG,
                                    bias=b2s_sb[:, m : m + 1],
                                )
                        if not last:
                            agin = agin_pool.tile(
                                [SLICE, CB], f16, name="agin", tag="agin"
                            )
                            agout = agout_pool.tile(
                                [D, CB],
                                f16,
                                name="agout",
                                tag="agout",
                                addr_space="Shared",
                            )
                            nc.sync.dma_start(
                                agin[:].rearrange("(m p) b -> p m b", p=P), own[:]
                            )
                            if not local_sim:
                                nc.gpsimd.collective_compute(
                                    "AllGather",
                                    BYPASS,
                                    replica_groups=RG,
                                    ins=[agin[:]],
                                    outs=[agout[:]],
                                )
                            nc.sync.dma_start(
                                s2_sb[:, :, cs],
                                agout[:].rearrange("(k p) b -> p k b", p=P),
                            )

    nc.compile()
    return nc


def _prep_inputs(data, W1, W2, b1, b2):
    """Host-side sharding/layout prep: slice, transpose, cast to fp16."""
    f16 = np.float16
    dataT = np.ascontiguousarray(data.T).astype(f16)  # [D, B], same on all cores
    in_maps = []
    for c in range(NCORES):
        hs = slice(c * SLICE, (c + 1) * SLICE)
        in_maps.append(
            {
                "w2t": np.ascontiguousarray(W2[hs, :].T).astype(f16),  # W2.T[:, H_c]
                "w2": np.ascontiguousarray(W2[:, hs]).astype(f16),
                "w1": np.ascontiguousarray(W1[:, hs]).astype(f16),
                "dataT": dataT,
                "b1s": np.ascontiguousarray(b1[hs]).astype(np.float32),
                "b2s": np.ascontiguousarray(b2[hs]).astype(np.float32),
                "b2f": np.ascontiguousarray(b2).astype(np.float32),
            }
        )
    return in_maps


def kernel(data, W1, W2, b1, b2, iterations):
    from concourse.bass_utils import run_bass_kernel_spmd

    data = np.asarray(data, dtype=np.float32)
    W1 = np.asarray(W1, dtype=np.float32)
    W2 = np.asarray(W2, dtype=np.float32)
    b1 = np.asarray(b1, dtype=np.float32)
    b2 = np.asarray(b2, dtype=np.float32)
    iters = int(iterations)

    assert data.shape == (B, D) and W1.shape == (D, D) and W2.shape == (D, D)

    if iters not in _CACHE:
        _CACHE[iters] = _build(iters)
    nc = _CACHE[iters]

    in_maps = _prep_inputs(data, W1, W2, b1, b2)
    res = run_bass_kernel_spmd(nc, in_maps, core_ids=list(range(NCORES)))

    s1T = np.concatenate([res.results[c]["s1_out"] for c in range(NCORES)], axis=0)
    s2T = np.concatenate([res.results[c]["s2_out"] for c in range(NCORES)], axis=0)
    return np.stack([np.ascontiguousarray(s1T.T), np.ascontiguousarray(s2T.T)])
